# revision 3
# baseline (speedup 1.0000x reference)
"""Trainium2 Bass kernel v6 for BertWithAdaThresholdLocContextPooling.

Data parallel over batch (2 per core x 8 cores). Key points:
  - mention rows of sequence_output/attention are gathered on the HOST
    (index application only); no device-side indirect DMA
  - one sync DMA ring in strict consumption order
  - ht normalization folded into the rs column scale (reciprocal applied
    via per-partition scalar at the rsc copy)
  - rs accumulation split: batch0 on DVE, batch1 on GpSimd (fused mult-add)
  - extractor processes hs chunks of both weights before rs chunks
"""

import sys

for _p in ("/opt/trn_rl_repo",):
    if _p not in sys.path:
        sys.path.insert(0, _p)

import numpy as np
import ml_dtypes

import concourse.bacc as bacc
import concourse.bass as bass
import concourse.mybir as mybir
from concourse.tile import TileContext
from concourse.bass_utils import run_bass_kernel_spmd

F32 = mybir.dt.float32
F8 = mybir.dt.float8e4
BF16 = mybir.dt.bfloat16
AF = mybir.ActivationFunctionType
ALU = mybir.AluOpType

B, L, HID = 16, 512, 768
HEADS, M = 12, 4
EMB, BLK, NER, NCLS = 768, 8, 6, 97
NCORES = 8
BPC = B // NCORES          # 2
CAT = 2 * HID + NER        # 1542
KCH = 12                   # full 128-row contraction chunks of cat
NEMB = EMB // 128          # 6
NL = L // 128              # 4
NBL = EMB * BLK // 128     # 48 classifier contraction chunks

# bf16 const pack [128, 158]:
#   selE [0:16,0:4] | selA [0:96,4:28] | w12 [0:12,28:29] | ones [0:128,29:30]
#   onesrow [0:1,30:158]
CBA_COLS = 158

_cache = {}


def _build_constants():
    selE = np.zeros((4 * M, 4), np.float32)
    for k in range(4 * M):
        selE[k, k // M] = 1.0
    selA = np.zeros((2 * M * HEADS, 2 * HEADS), np.float32)
    for i in range(2):
        for m in range(M):
            for h in range(HEADS):
                selA[i * M * HEADS + m * HEADS + h, i * HEADS + h] = 1.0 / M

    cba = np.zeros((128, CBA_COLS), ml_dtypes.bfloat16)
    cba[0:16, 0:4] = selE
    cba[0:96, 4:28] = selA
    cba[0:12, 28:29] = 1.0 / HEADS
    cba[0:128, 29:30] = 1.0
    cba[0:1, 30:158] = 1.0

    rys = np.zeros((128, BLK * 128), ml_dtypes.bfloat16)
    for y in range(BLK):
        for p in range(128):
            rys[(p // BLK) * BLK + y, y * 128 + p] = 1.0

    selb = np.zeros((1, 8), ml_dtypes.bfloat16)
    selb[0, 0:4] = np.array([1.0, 0.0, 1.0, 0.0])
    selb[0, 4:8] = np.array([0.0, 1.0, 0.0, 1.0])

    perm = np.empty(EMB * BLK, np.int64)
    for cch in range(NEMB):
        for y in range(BLK):
            for p in range(128):
                g = cch * 16 + p // BLK
                x = p % BLK
                perm[(cch * BLK + y) * 128 + p] = g * 64 + x * BLK + y
    return {"cba": cba, "rys": rys, "selb": selb, "perm": perm}


def _build_program():
    nc = bacc.Bacc("TRN2", target_bir_lowering=False, debug=False)

    seq_h = nc.dram_tensor("seq", [128, BPC * NL * HID], F8, kind="ExternalInput")
    atp_h = nc.dram_tensor("atp", [2 * M * HEADS, BPC * L], BF16, kind="ExternalInput")
    sg_h = nc.dram_tensor("sg", [4 * M, HID], BF16, kind="ExternalInput")
    cf_h = nc.dram_tensor("cf", [97, 9], F32, kind="ExternalInput")
    cba_h = nc.dram_tensor("cba", [128, CBA_COLS], BF16, kind="ExternalInput")
    whs_h = nc.dram_tensor("whs", [128, KCH * EMB], BF16, kind="ExternalInput")
    wts_h = nc.dram_tensor("wts", [128, KCH * EMB], BF16, kind="ExternalInput")
    wner_h = nc.dram_tensor("wner", [NER, 2 * EMB], BF16, kind="ExternalInput")
    wbias_h = nc.dram_tensor("wbias", [1, 2 * EMB + 8 + NCLS], BF16, kind="ExternalInput")
    rys_h = nc.dram_tensor("rys", [128, BLK * 128], BF16, kind="ExternalInput")
    wbs_h = nc.dram_tensor("wbs", [128, NBL * NCLS], BF16, kind="ExternalInput")
    out_h = nc.dram_tensor("logitsT", [NCLS, BPC], F32, kind="ExternalOutput")

    with TileContext(nc) as tc:
        with (
            tc.tile_pool(name="const", bufs=1) as cp,
            tc.tile_pool(name="data", bufs=1) as dp,
            tc.tile_pool(name="psbig", bufs=2, space="PSUM") as psb,
            tc.tile_pool(name="psea", bufs=2, space="PSUM") as pse,
            tc.tile_pool(name="pssm", bufs=2, space="PSUM") as pss,
        ):
            # ---- one sync DMA ring, strict consumption order ----
            atp = dp.tile([2 * M * HEADS, BPC * L], BF16)
            nc.sync.dma_start(atp[:], atp_h[:])
            sg = dp.tile([4 * M, HID], BF16)
            nc.sync.dma_start(sg[:], sg_h[:])
            cba = cp.tile([128, CBA_COLS], BF16)
            nc.sync.dma_start(cba[:], cba_h[:])
            cf = cp.tile([97, 9], F32)
            nc.sync.dma_start(cf[:], cf_h[:])
            wnerf = cp.tile([NER, 2 * EMB], BF16)
            nc.sync.dma_start(wnerf[:], wner_h[:])
            wbias = cp.tile([1, 2 * EMB + 8 + NCLS], BF16)
            nc.sync.dma_start(wbias[:], wbias_h[:])
            seqc = dp.tile([128, BPC * NL * HID], F8)
            whsf = cp.tile([128, KCH * EMB], BF16)
            nc.sync.dma_start(whsf[:, 0:6 * EMB], whs_h[:, 0:6 * EMB])
            nc.sync.dma_start(seqc[:], seq_h[:])
            seqt = [seqc[:, b * NL * HID:(b + 1) * NL * HID] for b in range(BPC)]
            nc.sync.dma_start(whsf[:, 6 * EMB:KCH * EMB],
                              whs_h[:, 6 * EMB:KCH * EMB])
            wtsf = cp.tile([128, KCH * EMB], BF16)
            nc.sync.dma_start(wtsf[:, 0:6 * EMB], wts_h[:, 0:6 * EMB])
            rys = cp.tile([128, BLK * 128], BF16)
            nc.sync.dma_start(rys[:], rys_h[:])
            nc.sync.dma_start(wtsf[:, 6 * EMB:KCH * EMB],
                              wts_h[:, 6 * EMB:KCH * EMB])
            wbs = cp.tile([128, NBL * NCLS], BF16)
            nc.sync.dma_start(wbs[:], wbs_h[:])

            ner4 = dp.tile([NER, 4], BF16)
            nc.vector.tensor_copy(ner4[:], cf[0:NER, 5:9])
            eye4 = cf[0:4, 0:4]
            bbc = cf[0:97, 4:5]
            selE = cba[0:16, 0:4]
            selA = cba[0:96, 4:28]
            w12 = cba[0:12, 28:29]
            ones = cba[0:128, 29:30]
            onesr = cba[0:1, 30:158]
            whn = wnerf[0:NER, 0:EMB]
            wtn = wnerf[0:NER, EMB:2 * EMB]
            bhr = wbias[0:1, 0:EMB]
            btr = wbias[0:1, EMB:2 * EMB]
            selbh = wbias[0:1, 2 * EMB:2 * EMB + 4]
            selbt = wbias[0:1, 2 * EMB + 4:2 * EMB + 8]
            bbr = wbias[0:1, 2 * EMB + 8:2 * EMB + 8 + NCLS]

            # ---- Exp act-table preload (Ln/Tanh loads overlap naturally) ----
            dum = dp.tile([1, 4], F32)
            nc.vector.memset(dum[:], 0.0)
            nc.scalar.activation(dum[:, 0:1], dum[0:1, 3:4], AF.Exp)

            # ---- attention pooling -> raw ht columns + 1/(sum+eps) ----
            at = [atp[:, b * L:(b + 1) * L] for b in range(BPC)]
            htc = []
            rcpp = []
            prds = []
            for b, veng in ((0, nc.vector), (1, nc.vector)):
                ps_eah = pse.tile([HEADS, L], F32, tag="ea")
                nc.tensor.matmul(ps_eah[:], lhsT=selA[:, 0:HEADS], rhs=at[b],
                                 start=True, stop=True)
                ps_eat = pse.tile([HEADS, L], F32, tag="ea")
                nc.tensor.matmul(ps_eat[:], lhsT=selA[:, HEADS:2 * HEADS],
                                 rhs=at[b], start=True, stop=True)
                eah = dp.tile([HEADS, L], F32, tag=f"eah{b}")
                if b == 0:
                    nc.vector.tensor_copy(eah[:], ps_eah[:])
                else:
                    nc.scalar.activation(eah[:], ps_eah[:], AF.Copy)
                prd = dp.tile([HEADS, L], BF16, tag=f"prd{b}")
                nc.vector.tensor_tensor(out=prd[:], in0=eah[:], in1=ps_eat[:],
                                        op=ALU.mult)
                prds.append(prd)
            for b in range(BPC):
                ps_htc = pss.tile([128, NL], F32, tag="sm")
                for c in range(NL):
                    nc.tensor.matmul(ps_htc[:, c:c + 1],
                                     lhsT=prds[b][:, c * 128:(c + 1) * 128],
                                     rhs=w12, start=True, stop=True)
                h = dp.tile([128, NL], F8, tag=f"htc{b}")
                with nc.allow_low_precision(reason="raw ht cols only feed rs; fp8 ok"):
                    nc.vector.tensor_copy(h[:], ps_htc[:])
                htc.append(h)
            for b in range(BPC):
                psum = dp.tile([HEADS, 1], BF16, tag=f"psum{b}")
                with nc.allow_low_precision(reason="ht-sum feeds 1/(x+eps); bf16 ok"):
                    nc.vector.reduce_sum(psum[:], prds[b][:],
                                         axis=mybir.AxisListType.X)
                ps_s = pss.tile([1, 1], F32, tag="sm")
                nc.tensor.matmul(ps_s[:], lhsT=w12, rhs=psum[:],
                                 start=True, stop=True)
                den = dp.tile([1, 1], F32, tag=f"den{b}")
                nc.vector.tensor_scalar_add(den[:], ps_s[:], 1e-5)
                rcp = dp.tile([1, 1], BF16, tag=f"rcp{b}")
                with nc.allow_low_precision(reason="bf16 1/(sum+eps) scale is ample"):
                    nc.vector.reciprocal(rcp[:], den[:])
                ps_r = pss.tile([128, 1], F32, tag="sm")
                nc.tensor.matmul(ps_r[:], lhsT=onesr, rhs=rcp[:],
                                 start=True, stop=True)
                rcb = dp.tile([128, 1], F32, tag=f"rcb{b}")
                nc.vector.tensor_copy(rcb[:], ps_r[:])
                rcpp.append(rcb)

            # ---- entity embeddings: logsumexp over mentions ----
            exps = dp.tile([4 * M, HID], BF16)
            nc.scalar.activation(exps[:], sg[:], AF.Exp)
            ps_e = psb.tile([4, HID], F32, tag="big")
            for n0, nl_ in ((0, 512), (512, 256)):
                nc.tensor.matmul(ps_e[:, n0:n0 + nl_], lhsT=selE,
                                 rhs=exps[:, n0:n0 + nl_], start=True, stop=True)
            ent = dp.tile([4, HID], F32)
            nc.scalar.activation(ent[:], ps_e[:], AF.Ln)
            ps_et = pss.tile([128, 4 * NEMB], F32, tag="sm")
            for c in range(NEMB):
                nc.tensor.transpose(ps_et[:, c * 4:(c + 1) * 4],
                                    ent[:, c * 128:(c + 1) * 128], eye4)
            entT = dp.tile([128, 4 * NEMB], BF16)
            nc.vector.tensor_copy(entT[:], ps_et[:])

            # ---- extractor: Wh hs chunks first (earliest data) ----
            ps_wh = psb.tile([4, EMB], F32, tag="big")
            for n0, nl_ in ((0, 512), (512, 256)):
                for j in range(NEMB):
                    nc.tensor.matmul(ps_wh[:, n0:n0 + nl_],
                                     lhsT=entT[:, j * 4:(j + 1) * 4],
                                     rhs=whsf[:, j * EMB + n0:j * EMB + n0 + nl_],
                                     start=(j == 0), stop=False)

            # rs columns via col-form matmuls (both batches)
            ps_rs = pss.tile([128, NEMB * BPC], F32, tag="sm")
            for b in range(BPC):
                for d in range(NEMB):
                    for c in range(NL):
                        nc.tensor.matmul(
                            ps_rs[:, d * BPC + b:d * BPC + b + 1],
                            lhsT=seqt[b][:, c * HID + d * 128:c * HID + (d + 1) * 128],
                            rhs=htc[b][:, c:c + 1],
                            start=(c == 0), stop=(c == NL - 1))
            rsc = dp.tile([128, 4 * NEMB], BF16)
            for b in range(BPC):
                nc.vector.tensor_scalar_mul(
                    rsc[:].rearrange("p (r b m) -> p r b m", r=NEMB, b=BPC)
                    [:, :, b, :],
                    ps_rs[:].rearrange("p (r b) -> p r b", r=NEMB)[:, :, b]
                    .unsqueeze(2).broadcast_to([128, NEMB, 2]),
                    rcpp[b][:, 0:1])

            ps_wt = psb.tile([4, EMB], F32, tag="big")
            for n0, nl_ in ((0, 512), (512, 256)):
                for j in range(NEMB):
                    nc.tensor.matmul(ps_wt[:, n0:n0 + nl_],
                                     lhsT=entT[:, j * 4:(j + 1) * 4],
                                     rhs=wtsf[:, j * EMB + n0:j * EMB + n0 + nl_],
                                     start=(j == 0), stop=False)

            # finish extractor: rs chunks + ner + bias, then per-nblock
            # tanh and transposes (overlap Scalar with remaining PE work)
            t4h = dp.tile([4, EMB], F32)
            t4t = dp.tile([4, EMB], F32)
            ps_a = pss.tile([128, 4 * NEMB], F32, tag="sm")
            ps_b2 = pss.tile([128, 4 * NEMB], F32, tag="sm")
            for ps_w, ws, wn, selb, br, t4, ps_t in (
                    (ps_wh, whsf, whn, selbh, bhr, t4h, ps_a),
                    (ps_wt, wtsf, wtn, selbt, btr, t4t, ps_b2)):
                for n0, nl_ in ((0, 512), (512, 256)):
                    for j in range(NEMB):
                        nc.tensor.matmul(
                            ps_w[:, n0:n0 + nl_],
                            lhsT=rsc[:, j * 4:(j + 1) * 4],
                            rhs=ws[:, (NEMB + j) * EMB + n0:
                                    (NEMB + j) * EMB + n0 + nl_],
                            start=False, stop=False)
                    nc.tensor.matmul(ps_w[:, n0:n0 + nl_], lhsT=ner4[:],
                                     rhs=wn[:, n0:n0 + nl_],
                                     start=False, stop=False)
                    nc.tensor.matmul(ps_w[:, n0:n0 + nl_], lhsT=selb,
                                     rhs=br[:, n0:n0 + nl_],
                                     start=False, stop=True)
                    nc.scalar.activation(t4[:, n0:n0 + nl_], ps_w[:, n0:n0 + nl_],
                                         AF.Tanh)
            for t4, ps_t in ((t4h, ps_a), (t4t, ps_b2)):
                for c in range(NEMB):
                    nc.tensor.transpose(ps_t[:, c * 4:(c + 1) * 4],
                                        t4[:, c * 128:(c + 1) * 128], eye4)
            h2t = dp.tile([128, 4 * NEMB], BF16)
            nc.vector.tensor_copy(
                h2t[:].rearrange("p (c b) -> p c b", c=NEMB)[:, :, 0:4:2],
                ps_a[:].rearrange("p (c b) -> p c b", c=NEMB)[:, :, 0:4:2])
            nc.vector.tensor_copy(
                h2t[:].rearrange("p (c b) -> p c b", c=NEMB)[:, :, 1:4:2],
                ps_b2[:].rearrange("p (c b) -> p c b", c=NEMB)[:, :, 1:4:2])

            # ---- grouped bilinear + classifier ----
            ps_t2x = pss.tile([128, BLK * NEMB * BPC], F32, tag="sm")
            tscols = h2t[:].rearrange("p (c b) -> p c b", c=NEMB)[:, :, 1:4:2]
            for y in range(BLK):
                nc.tensor.matmul(
                    ps_t2x[:, y * 12:(y + 1) * 12]
                    .rearrange("p (c b) -> p c b", c=NEMB),
                    lhsT=rys[:, y * 128:(y + 1) * 128],
                    rhs=tscols, start=True, stop=True)
            blt = dp.tile([128, NEMB * 16], BF16)
            for c in range(NEMB):
                nc.vector.tensor_tensor(
                    out=blt[:, c * 16:(c + 1) * 16]
                    .rearrange("p (y b) -> p y b", y=BLK),
                    in0=h2t[:, c * 4:c * 4 + 4:2].unsqueeze(1)
                        .broadcast_to([128, BLK, 2]),
                    in1=ps_t2x[:].rearrange("p (y c b) -> p y c b", y=BLK, c=NEMB)
                    [:, :, c, :],
                    op=ALU.mult)
            ps_l = pss.tile([NCLS, BPC], F32, tag="sm")
            for c in range(NEMB):
                for y in range(BLK):
                    k = c * BLK + y
                    nc.tensor.matmul(ps_l[:], lhsT=wbs[:, k * NCLS:(k + 1) * NCLS],
                                     rhs=blt[:, c * 16 + y * 2:c * 16 + y * 2 + 2],
                                     start=(k == 0), stop=False)
            nc.tensor.matmul(ps_l[:], lhsT=bbr, rhs=cba[0:1, 29:31],
                             start=False, stop=True)
            lg = dp.tile([NCLS, BPC], F32)
            nc.vector.tensor_copy(lg[:], ps_l[:])
            nc.sync.dma_start(out_h[:], lg[:])

    nc.finalize()
    return nc


def _get_program():
    if "nc" not in _cache:
        _cache["consts"] = _build_constants()
        _cache["nc"] = _build_program()
    return _cache["nc"], _cache["consts"]


def kernel(sequence_output, attention, entity_pos, hs_ner_tags, ts_ner_tags,
           Wh, bh, Wt, bt, Wb, bb):
    nc, c = _get_program()

    seqf = np.asarray(sequence_output, dtype=np.float32)
    seq = seqf.astype(ml_dtypes.float8_e4m3)
    attn = np.asarray(attention, dtype=np.float32)
    pos = np.asarray(entity_pos).astype(np.int64)
    nh = np.asarray(hs_ner_tags, dtype=np.float32)
    nt = np.asarray(ts_ner_tags, dtype=np.float32)
    whT = np.ascontiguousarray(np.asarray(Wh, dtype=np.float32).T).astype(ml_dtypes.bfloat16)
    wtT = np.ascontiguousarray(np.asarray(Wt, dtype=np.float32).T).astype(ml_dtypes.bfloat16)
    wbT = np.ascontiguousarray(np.asarray(Wb, dtype=np.float32).T)[c["perm"]]

    def sbuf_image(w):
        return np.ascontiguousarray(
            w[0:KCH * 128].reshape(KCH, 128, EMB).transpose(1, 0, 2)
            .reshape(128, KCH * EMB))

    whs = sbuf_image(whT)
    wts = sbuf_image(wtT)
    wner = np.concatenate([whT[KCH * 128:CAT], wtT[KCH * 128:CAT]],
                          axis=1).astype(ml_dtypes.bfloat16)
    wner = np.ascontiguousarray(wner)
    wbs = wbT.reshape(NBL, 128, NCLS).transpose(1, 0, 2).reshape(128, NBL * NCLS)
    wbs = np.ascontiguousarray(wbs.astype(ml_dtypes.bfloat16))

    wbias = np.zeros((1, 2 * EMB + 8 + NCLS), ml_dtypes.bfloat16)
    wbias[0, 0:EMB] = np.asarray(bh, np.float32)
    wbias[0, EMB:2 * EMB] = np.asarray(bt, np.float32)
    wbias[0, 2 * EMB:2 * EMB + 8] = c["selb"][0]
    wbias[0, 2 * EMB + 8:] = np.asarray(bb, np.float32)
    cf0 = np.zeros((97, 9), np.float32)
    cf0[0:4, 0:4] = np.eye(4)
    cf0[0:97, 4] = np.asarray(bb, np.float32)

    in_maps = []
    hh = np.arange(HEADS)
    for core in range(NCORES):
        b0 = core * BPC
        pc = pos[b0:b0 + BPC]                                # [2,2,M]
        # host-side gathers (index application only; math stays on device)
        atp = np.empty((2 * M * HEADS, BPC * L), np.float32)
        sgr = np.empty((4 * M, HID), np.float32)
        for b in range(BPC):
            for i in range(2):
                for m in range(M):
                    l = pc[b, i, m] + 1
                    atp[i * M * HEADS + m * HEADS + hh, b * L:(b + 1) * L] = \
                        attn[b0 + b, :, l, :]
                    sgr[b * 2 * M + i * M + m] = seqf[b0 + b, l]
        ner = np.stack([nh[b0], nt[b0], nh[b0 + 1], nt[b0 + 1]], axis=1)
        cfc = cf0.copy()
        cfc[0:NER, 5:9] = ner
        im = {
            "seq": np.ascontiguousarray(
                seq[b0:b0 + BPC].reshape(BPC, NL, 128, HID)
                .transpose(2, 0, 1, 3).reshape(128, BPC * NL * HID)),
            "atp": atp.astype(ml_dtypes.bfloat16),
            "sg": sgr.astype(ml_dtypes.bfloat16),
            "cf": cfc, "cba": c["cba"],
            "whs": whs, "wts": wts, "wner": wner, "wbias": wbias,
            "rys": c["rys"], "wbs": wbs,
        }
        in_maps.append(im)

    res = run_bass_kernel_spmd(nc, in_maps, core_ids=list(range(NCORES)))
    _cache["last_res"] = res
    out = np.empty((B, NCLS), np.float32)
    for core in range(NCORES):
        out[core * BPC:(core + 1) * BPC] = res.results[core]["logitsT"].T
    return out
